# revision 10
# baseline (speedup 1.0000x reference)
"""Trainium2 Bass kernel for the SCAN-style t2i contrastive loss.

Math restructure (vs reference):
  - softmax denominator over regions cancels in the cosine similarity -> never computed
  - num[i,jl]  = sum_r E[ir,jl] * B[ir,jl]          (B = raw attention, pre-LeakyReLU)
  - wn^2[i,jl] = E^T G_i E  via H = blockdiag(G) @ E (G_i = im_i @ im_i^T Gram, caption-independent)
  - word mask baked into caption features host-side (masked word rows = 0)

Sharding: 32 captions per core (8 cores), images replicated.
Layout: partition = (image,region) in groups of 108 rows (3 images), free = (caption,word) = 1600.
"""

import os
import sys

for _p in ("/opt/trn_rl_repo", "/root/.axon_site/_ro/trn_rl_repo"):
    if os.path.isdir(_p) and _p not in sys.path:
        sys.path.insert(0, _p)

import ml_dtypes
import numpy as np

import concourse.bass as bass
import concourse.mybir as mybir
import concourse.tile as tile
from concourse.bass_utils import run_bass_kernel_spmd

F32 = mybir.dt.float32
BF16 = mybir.dt.bfloat16
AF = mybir.ActivationFunctionType
ALU = mybir.AluOpType

N, R, L, D = 256, 36, 50, 256
NCORES = 8
JCAP = N // NCORES          # 32 captions per core
JL = JCAP * L               # 1600
PG = 108                    # partition rows per group = 3 images * 36 regions
NIMG_G = 3
NG = (N + NIMG_G - 1) // NIMG_G   # 86 compute groups (last has 1 image)
GSH = 11                    # padded groups per image shard (8*11=88 >= 86)
SHARD = GSH * PG            # 1188 (i,r) columns shipped per core
IRPAD = NCORES * SHARD      # 9504 padded (i,r) rows after AllGather
KC = 2                      # D = 2 chunks of 128
CHUNKS = [(0, 512), (512, 512), (1024, 512), (1536, 64)]
PQCH = [(0, 256), (256, 256), (512, 256), (768, 256),
        (1024, 256), (1280, 256), (1536, 64)]
WIN = 4                     # groups per PQ window (32-aligned psum slots)
LSM, LLSE, MARGIN, EPS = 9.0, 6.0, 0.2, 1e-8

_NC_CACHE = {}


def _patched_drain_and_barrier(self, tick_clock, wait_clock):
    """Walrus in this env rejects >1 sync-wait per instruction; split the
    Tile tail-drain's global-clock waits onto one DVE memset each."""
    gc = tick_clock.global_clock
    sems = self.sems.allocated()
    scratch = self.nc._drain_scratch
    for proc, sem in sems.items():
        tick = gc[proc]
        if tick <= 0:
            continue
        val = tick * 16 if sem.name.startswith("DMA") else tick
        self.nc.vector.memset(scratch[:, :], 0.0).wait_op(sem, val, "sem-ge")
    self.nc.sync.drain()
    self.nc.all_engine_barrier()
    assert self.sems is not None
    popped = self.nc._tile_sem_poison_stack.pop()
    assert popped is self._sem_poison
    self.nc.clear_and_free_semaphores(list(self.sems.allocated().values()))
    self.nc.all_engine_barrier()


tile.TileContext._drain_and_barrier = _patched_drain_and_barrier


# ---------------------------------------------------------------------------
# run_bass_kernel_spmd re-jits a fresh closure every call, so XLA's in-memory
# executable cache (keyed on MLIR module object identity) misses and
# neuronx_cc_hook re-runs walrus BIR-verify + DVE table gen (~0.6s) per
# dispatch for an identical HLO. Memoize the hook on the HLO content hash:
# same bytes in -> same NEFF custom-call out.
import hashlib

import concourse.bass2jax as bass2jax

_ORIG_CC_HOOK = bass2jax.neuronx_cc_hook
_CC_HOOK_MEMO = {}


def _memo_cc_hook(code, code_format, platform_version, file_prefix):
    key = None
    if isinstance(code, bytes):
        key = hashlib.sha256(code_format + b"\x00" + code).digest()
        if key in _CC_HOOK_MEMO:
            return _CC_HOOK_MEMO[key]
    r = _ORIG_CC_HOOK(code, code_format, platform_version, file_prefix)
    if key is not None:
        _CC_HOOK_MEMO[key] = r
    return r


bass2jax.neuronx_cc_hook = _memo_cc_hook


def _split_multiwaits(nc):
    """This walrus build accepts at most one sync-wait per instruction.
    Rewrite the serialized BIR: move extra waits onto EventSemaphore
    carriers inserted immediately before the instruction (same engine,
    order preserved, so semantics are identical)."""
    import orjson
    d = orjson.loads(nc.to_json_bytes())
    uid = [0]
    for f in d["functions"]:
        for b in f["blocks"]:
            out = []
            for inst in b["instructions"]:
                si = inst.get("sync_info") or {}
                waits = si.get("on_wait") or []
                if len(waits) > 1:
                    for wnode in waits[:-1]:
                        uid[0] += 1
                        out.append({
                            "debug": inst.get("debug"),
                            "engine": inst["engine"],
                            "ins": [], "outs": [],
                            "name": f"wsplit_{uid[0]}",
                            "opcode": "EventSemaphore",
                            "sync_info": {"on_update": [], "on_wait": [wnode]},
                        })
                    si["on_wait"] = [waits[-1]]
                out.append(inst)
            b["instructions"] = out
    return orjson.dumps(d)


def _bcast_inner(ap, n):
    """Append a stride-0 inner axis of length n (free-dim broadcast)."""
    return bass.AP(tensor=ap.tensor, offset=ap.offset, ap=[*ap.ap, [0, n]])


def _bcast_part(ap, p):
    """Replace partition axis with stride-0 broadcast of length p (DMA use)."""
    return bass.AP(tensor=ap.tensor, offset=ap.offset, ap=[[0, p], *ap.ap[1:]])


def _build_nc():
    nc = bass.Bass("TRN2", target_bir_lowering=False)
    nc._drain_scratch = nc.sbuf_tensor("drainscr", [1, 1], F32).__enter__()

    imt_d = nc.dram_tensor("imt", [KC, 128, SHARD], BF16, kind="ExternalInput")
    st_d = nc.dram_tensor("st", [KC, 128, JL], BF16, kind="ExternalInput")
    gmask_d = nc.dram_tensor("gmask", [PG, PG], BF16, kind="ExternalInput")
    onesb_d = nc.dram_tensor("onesb", [PG, NIMG_G], BF16, kind="ExternalInput")
    maskjl_d = nc.dram_tensor("maskjl", [1, JL], BF16, kind="ExternalInput")
    lse_d = nc.dram_tensor("lseout", [N, JCAP], F32, kind="ExternalOutput")

    with tile.TileContext(nc) as tc:
        with (
            tc.tile_pool(name="persist", bufs=1) as pp,
            tc.tile_pool(name="work", bufs=int(os.environ.get("K_WPB", "2"))) as wp,
            tc.tile_pool(name="fb", bufs=WIN + 1) as fbp,
            tc.tile_pool(name="scr1", bufs=1) as scrp,
            tc.tile_pool(name="post", bufs=1) as postp,
            tc.tile_pool(name="small", bufs=3) as sp,
            tc.tile_pool(name="bps", bufs=1, space="PSUM") as bpool,
            tc.tile_pool(name="hps", bufs=2, space="PSUM") as hpool,
            tc.tile_pool(name="pqps", bufs=2, space="PSUM") as pqpool,
        ):
            imt = pp.tile([128, KC, IRPAD], BF16)
            st = pp.tile([128, KC, JL], BF16)
            gmask = pp.tile([PG, PG], BF16)
            onesb = pp.tile([PG, NIMG_G], BF16)
            g_all = pp.tile([PG, NG, PG], BF16)
            pq_all = pp.tile([128, 2, 2, JL], F32)   # [row, itile, P/Q, jl]
            cn_b = pp.tile([128, JL], F32)
            mask_b = pp.tile([128, JL], BF16)

            # images arrive sharded 1/8 per core; AllGather over the chip
            # interconnect instead of shipping 8 replicas through the host
            # tunnel (that replication dominated dispatch wall time).
            with tc.tile_pool(name="ccdr", bufs=1, space="DRAM") as ccp:
                inb = ccp.tile([KC, 128, SHARD], BF16)
                outb = ccp.tile([NCORES, KC, 128, SHARD], BF16)
                nc.gpsimd.dma_start(inb[:, :, :], imt_d[:, :, :])
                nc.gpsimd.collective_compute(
                    "AllGather", ALU.bypass,
                    replica_groups=[list(range(NCORES))],
                    ins=[inb.opt()], outs=[outb.opt()])
                for r in range(NCORES):
                    for kc in range(KC):
                        nc.sync.dma_start(
                            out=imt[:, kc, r * SHARD:(r + 1) * SHARD],
                            in_=outb[r, kc])
            for kc in range(KC):
                nc.sync.dma_start(out=st[:, kc, :], in_=st_d[kc])
            nc.sync.dma_start(out=gmask, in_=gmask_d[:, :])
            nc.sync.dma_start(out=onesb, in_=onesb_d[:, :])
            nc.sync.dma_start(out=mask_b, in_=_bcast_part(maskjl_d[0:1, :], 128))

            # ---- caption word norms cn[jl] = ||s_word||  (from masked sT) ----
            cn_sb = pp.tile([1, JL], F32)
            sq0 = postp.tile([128, JL], F32, tag="pA")
            sq1 = postp.tile([128, JL], F32, tag="pB")
            nc.vector.tensor_mul(sq0, st[:, 0, :], st[:, 0, :])
            nc.vector.tensor_mul(sq1, st[:, 1, :], st[:, 1, :])
            ones128 = pp.tile([128, 1], F32)
            nc.vector.memset(ones128, 1.0)
            for c0, cw in CHUNKS:
                cnps = pqpool.tile([1, 512], F32, tag="pq")
                nc.tensor.matmul(cnps[:, :cw], ones128, sq0[:, c0:c0 + cw],
                                 start=True, stop=False)
                nc.tensor.matmul(cnps[:, :cw], ones128, sq1[:, c0:c0 + cw],
                                 start=False, stop=True)
                nc.scalar.sqrt(cn_sb[0:1, c0:c0 + cw], cnps[:, :cw])
            # keep masked columns finite: cn = max(cn, 1e-6)
            nc.vector.tensor_scalar_max(cn_sb, cn_sb, 1e-6)
            with tc.tile_pool(name="drbnc", bufs=1, space="DRAM") as drp:
                cn_dr = drp.tile([1, JL], F32)
                nc.sync.dma_start(out=cn_dr[:, :], in_=cn_sb[:, :])
                nc.sync.dma_start(out=cn_b, in_=_bcast_part(cn_dr[0:1, :], 128))

            # ---- per-group Gram matrices (block-diag masked) ----
            for g in range(NG):
                gsl = slice(g * PG, (g + 1) * PG)
                gps = pqpool.tile([PG, PG], F32, tag="pq")
                for kc in range(KC):
                    nc.tensor.matmul(gps, imt[:, kc, gsl], imt[:, kc, gsl],
                                     start=(kc == 0), stop=(kc == KC - 1))
                nc.vector.tensor_mul(g_all[:, g, :], gps, gmask)

            # ---- main pipeline: windows of 4 groups ----
            for w in range((NG + WIN - 1) // WIN):
                gset = [g for g in range(w * WIN, min((w + 1) * WIN, NG))]
                fts = {}
                for g in gset:
                    gsl = slice(g * PG, (g + 1) * PG)
                    bps = bpool.tile([PG, JL], F32, tag="B")
                    for c0, cw in CHUNKS:
                        for kc in range(KC):
                            nc.tensor.matmul(bps[:, c0:c0 + cw], imt[:, kc, gsl],
                                             st[:, kc, c0:c0 + cw],
                                             start=(kc == 0), stop=(kc == KC - 1))

                    Rt = wp.tile([PG, JL], BF16, tag="R")
                    Bc = wp.tile([PG, JL], BF16, tag="Bc")
                    nc.scalar.activation(Rt, bps, AF.Lrelu, alpha=0.1)   # ACT
                    _bceng = nc.scalar.copy if os.environ.get("K_BC", "v") == "s" else nc.vector.tensor_copy
                    _bceng(Bc, bps)

                    St = wp.tile([PG, JL], BF16, tag="S")
                    nc.scalar.square(St, Rt)                             # ACT
                    n2 = sp.tile([PG, JCAP], F32, tag="n2")
                    nc.vector.tensor_reduce(
                        n2, St.rearrange("p (j l) -> p j l", l=L),
                        axis=mybir.AxisListType.X, op=ALU.add)           # DVE
                    n1 = sp.tile([PG, JCAP], F32, tag="n1")
                    nc.scalar.sqrt(n1, n2)                               # ACT small
                    nc.vector.tensor_scalar_add(n1, n1, EPS)             # DVE small
                    inv = sp.tile([PG, JCAP], F32, tag="inv")
                    nc.vector.reciprocal(inv, n1)                        # DVE small

                    M1 = wp.tile([PG, JL], BF16, tag="M1")
                    _m1eng = nc.vector if os.environ.get("K_M1", "g") == "v" else nc.gpsimd
                    _m1eng.tensor_tensor(
                        M1.rearrange("p (j l) -> p j l", l=L),
                        Rt.rearrange("p (j l) -> p j l", l=L),
                        _bcast_inner(inv[:, :], L), op=ALU.mult)
                    Et = wp.tile([PG, JL], BF16, tag="E")
                    nc.scalar.activation(Et, M1, AF.Exp, scale=LSM)      # ACT

                    F1 = fbp.tile([PG, JL], BF16, tag="F1")
                    _f1eng = nc.vector if os.environ.get("K_F1", "g") == "v" else nc.gpsimd
                    _f1eng.tensor_mul(F1, Et, Bc)
                    F2 = fbp.tile([PG, JL], BF16, tag="F2")
                    for c0, cw in CHUNKS:
                        hps = hpool.tile([PG, 512], F32, tag="H")
                        nc.tensor.matmul(hps[:, :cw], g_all[:, g, :],
                                         Et[:, c0:c0 + cw], start=True, stop=True)
                        nc.vector.tensor_mul(F2[:, c0:c0 + cw],
                                             Et[:, c0:c0 + cw], hps[:, :cw])  # DVE
                    fts[g] = (F1, F2)

                # PQ reduce for the window: 32-aligned psum slots per group
                scr = scrp.tile([99, 2, JL], F32, tag="scr")
                for c0, cw in PQCH:
                    pqa = pqpool.tile([99, 2, 256], F32, tag="pq")
                    for qi, g in enumerate(gset):
                        for pqi in range(2):
                            nc.tensor.matmul(
                                pqa[32 * qi:32 * qi + NIMG_G, pqi, :cw],
                                onesb, fts[g][pqi][:, c0:c0 + cw],
                                start=True, stop=True,
                                tile_position=(0, 32 * qi))
                    nc.scalar.copy(scr[:, :, c0:c0 + cw], pqa[:, :, :cw])  # ACT
                # scatter rows: image 3g+b lives at scr[32*(g%WIN)+b]
                for qi, g in enumerate(gset):
                    nimg = NIMG_G if g < NG - 1 else N - NIMG_G * (NG - 1)
                    b = 0
                    while b < nimg:
                        row = g * NIMG_G + b
                        it, r0 = row // 128, row % 128
                        nrun = min(nimg - b, 128 - r0)
                        nc.sync.dma_start(
                            out=pq_all[r0:r0 + nrun, it, :, :],
                            in_=scr[32 * qi + b:32 * qi + b + nrun, :, :])
                        b += nrun

            # ---- post stage: sim -> exp -> masked LSE ----
            for it in range(2):
                qa = postp.tile([128, JL], F32, tag="pA")
                qb = postp.tile([128, JL], F32, tag="pB")
                nc.scalar.sqrt(qa, pq_all[:, it, 1, :])              # q = sqrt(Q^2)
                nc.vector.tensor_mul(qa, qa, cn_b)                   # q*cn in place
                nc.vector.reciprocal(qb, qa)                         # 1/(q*cn)
                nc.vector.tensor_mul(qb, pq_all[:, it, 0, :], qb)    # sim in place
                nc.scalar.activation(qa, qb, AF.Exp, scale=LLSE)
                nc.vector.tensor_mul(qa, qa, mask_b)                 # masked exp
                ssum = sp.tile([128, JCAP], F32, tag="ssum")
                nc.vector.tensor_reduce(
                    ssum, qa.rearrange("p (j l) -> p j l", l=L),
                    axis=mybir.AxisListType.X, op=ALU.add)
                lse = sp.tile([128, JCAP], F32, tag="lse")
                nc.scalar.activation(lse, ssum, AF.Ln)
                nc.sync.dma_start(out=lse_d[it * 128:(it + 1) * 128, :], in_=lse)

    return nc


def kernel(im, s, cap_lens):
    im = np.asarray(im, np.float32)
    s = np.asarray(s, np.float32)
    cap_lens = np.asarray(cap_lens, np.int32)

    # host prep: mask padded words, transpose to (d, rows), pad ir, cast bf16
    wmask = (np.arange(L)[None, :] < cap_lens[:, None])          # (N, L)
    s_m = s * wmask[:, :, None].astype(np.float32)
    imt_full = np.zeros((D, IRPAD), np.float32)
    imt_full[:, :N * R] = im.reshape(N * R, D).T
    imt_bf = imt_full.astype(ml_dtypes.bfloat16)

    gmask = np.kron(np.eye(NIMG_G, dtype=np.float32),
                    np.ones((R, R), np.float32)).astype(ml_dtypes.bfloat16)
    onesb = np.kron(np.eye(NIMG_G, dtype=np.float32),
                    np.ones((R, 1), np.float32)).astype(ml_dtypes.bfloat16)

    in_maps = []
    for c in range(NCORES):
        js = slice(c * JCAP, (c + 1) * JCAP)
        stc = s_m[js].reshape(JL, D).T                            # (256, 1600)
        stc = np.ascontiguousarray(
            stc.reshape(KC, 128, JL)).astype(ml_dtypes.bfloat16)
        mjl = wmask[js].reshape(1, JL).astype(ml_dtypes.bfloat16)
        imtc = np.ascontiguousarray(
            imt_bf[:, c * SHARD:(c + 1) * SHARD].reshape(KC, 128, SHARD))
        in_maps.append({"imt": imtc, "st": stc, "gmask": gmask,
                        "onesb": onesb, "maskjl": mjl})

    _NC_CACHE["in_maps"] = in_maps
    if "nc" not in _NC_CACHE:
        nc = _build_nc()
        patched = _split_multiwaits(nc)
        nc.to_json_bytes = lambda: patched
        _NC_CACHE["nc"] = nc
    res = run_bass_kernel_spmd(_NC_CACHE["nc"], in_maps,
                               core_ids=list(range(NCORES)))
    outs = res.results if hasattr(res, "results") else res

    scores = np.concatenate(
        [o["lseout"].astype(np.float64) / LLSE for o in outs], axis=1)  # (256,256)

    d = np.diag(scores)
    cs = np.maximum(MARGIN + scores - d[:, None], 0.0)
    ci = np.maximum(MARGIN + scores - d[None, :], 0.0)
    np.fill_diagonal(cs, 0.0)
    np.fill_diagonal(ci, 0.0)
    return np.float32(cs.sum() + ci.sum())



# revision 16
# speedup vs baseline: 3.4015x; 3.4015x over previous
"""Trainium2 Bass kernel for the SCAN-style t2i contrastive loss.

Math restructure (vs reference):
  - softmax denominator over regions cancels in the cosine similarity -> never computed
  - num[i,jl]  = sum_r E[ir,jl] * B[ir,jl]          (B = raw attention, pre-LeakyReLU)
  - wn^2[i,jl] = E^T G_i E  via H = blockdiag(G) @ E (G_i = im_i @ im_i^T Gram, caption-independent)
  - word mask baked into caption features host-side (masked word rows = 0)

Sharding: 32 captions per core (8 cores), images replicated.
Layout: partition = (image,region) in groups of 108 rows (3 images), free = (caption,word) = 1600.
"""

import os
import sys

for _p in ("/opt/trn_rl_repo", "/root/.axon_site/_ro/trn_rl_repo"):
    if os.path.isdir(_p) and _p not in sys.path:
        sys.path.insert(0, _p)

import ml_dtypes
import numpy as np

import concourse.bass as bass
import concourse.mybir as mybir
import concourse.tile as tile
from concourse.bass_utils import run_bass_kernel_spmd

F32 = mybir.dt.float32
BF16 = mybir.dt.bfloat16
F8 = mybir.dt.float8e4
AF = mybir.ActivationFunctionType
ALU = mybir.AluOpType

N, R, L, D = 256, 36, 50, 256
NCORES = 8
JCAP = N // NCORES          # 32 captions per core
JL = JCAP * L               # 1600
PG = 108                    # partition rows per group = 3 images * 36 regions
NIMG_G = 3
NG = (N + NIMG_G - 1) // NIMG_G   # 86 compute groups (last has 1 image)
GSH = 11                    # padded groups per image shard (8*11=88 >= 86)
SHARD = GSH * PG            # 1188 (i,r) columns shipped per core
IRPAD = NCORES * SHARD      # 9504 padded (i,r) rows after AllGather
KC = 2                      # D = 2 chunks of 128
CHUNKS = [(0, 512), (512, 512), (1024, 512), (1536, 64)]
PQCH = [(0, 256), (256, 256), (512, 256), (768, 256),
        (1024, 256), (1280, 256), (1536, 64)]
WIN = 4                     # groups per PQ window (32-aligned psum slots)
LSM, LLSE, MARGIN, EPS = 9.0, 6.0, 0.2, 1e-8

_NC_CACHE = {}


def _patched_drain_and_barrier(self, tick_clock, wait_clock):
    """Walrus in this env rejects >1 sync-wait per instruction; split the
    Tile tail-drain's global-clock waits onto one DVE memset each."""
    gc = tick_clock.global_clock
    sems = self.sems.allocated()
    scratch = self.nc._drain_scratch
    for proc, sem in sems.items():
        tick = gc[proc]
        if tick <= 0:
            continue
        val = tick * 16 if sem.name.startswith("DMA") else tick
        self.nc.vector.memset(scratch[:, :], 0.0).wait_op(sem, val, "sem-ge")
    self.nc.sync.drain()
    self.nc.all_engine_barrier()
    assert self.sems is not None
    popped = self.nc._tile_sem_poison_stack.pop()
    assert popped is self._sem_poison
    self.nc.clear_and_free_semaphores(list(self.sems.allocated().values()))
    self.nc.all_engine_barrier()


tile.TileContext._drain_and_barrier = _patched_drain_and_barrier


# ---------------------------------------------------------------------------
# run_bass_kernel_spmd re-jits a fresh closure every call, so XLA's in-memory
# executable cache (keyed on MLIR module object identity) misses and
# neuronx_cc_hook re-runs walrus BIR-verify + DVE table gen (~0.6s) per
# dispatch for an identical HLO. Memoize the hook on the HLO content hash:
# same bytes in -> same NEFF custom-call out.
import hashlib

import concourse.bass2jax as bass2jax

_ORIG_CC_HOOK = bass2jax.neuronx_cc_hook
_CC_HOOK_MEMO = {}


def _cc_hook_key(code, code_format):
    """Hash the HLO with call-unique fields (module id counter, jit call-site
    line in the stack-frame index) zeroed; all semantic content remains."""
    try:
        import libneuronxla.proto.hlo_pb2 as _hp
        p = _hp.HloModuleProto.FromString(code)
        p.id = 0
        p.ClearField("stack_frame_index")
        return hashlib.sha256(code_format + b"\x00" + p.SerializeToString()).digest()
    except Exception:
        return hashlib.sha256(code_format + b"\x00" + code).digest()


def _memo_cc_hook(code, code_format, platform_version, file_prefix):
    key = None
    if isinstance(code, bytes):
        key = _cc_hook_key(code, code_format)
        if key in _CC_HOOK_MEMO:
            return _CC_HOOK_MEMO[key]
    r = _ORIG_CC_HOOK(code, code_format, platform_version, file_prefix)
    if key is not None:
        _CC_HOOK_MEMO[key] = r
    return r


bass2jax.neuronx_cc_hook = _memo_cc_hook


def _split_multiwaits(nc):
    """This walrus build accepts at most one sync-wait per instruction.
    Rewrite the serialized BIR: move extra waits onto EventSemaphore
    carriers inserted immediately before the instruction (same engine,
    order preserved, so semantics are identical)."""
    import orjson
    d = orjson.loads(nc.to_json_bytes())
    uid = [0]
    for f in d["functions"]:
        for b in f["blocks"]:
            out = []
            for inst in b["instructions"]:
                si = inst.get("sync_info") or {}
                waits = si.get("on_wait") or []
                if len(waits) > 1:
                    for wnode in waits[:-1]:
                        uid[0] += 1
                        out.append({
                            "debug": inst.get("debug"),
                            "engine": inst["engine"],
                            "ins": [], "outs": [],
                            "name": f"wsplit_{uid[0]}",
                            "opcode": "EventSemaphore",
                            "sync_info": {"on_update": [], "on_wait": [wnode]},
                        })
                    si["on_wait"] = [waits[-1]]
                out.append(inst)
            b["instructions"] = out
    return orjson.dumps(d)


def _bcast_inner(ap, n):
    """Append a stride-0 inner axis of length n (free-dim broadcast)."""
    return bass.AP(tensor=ap.tensor, offset=ap.offset, ap=[*ap.ap, [0, n]])


def _bcast_part(ap, p):
    """Replace partition axis with stride-0 broadcast of length p (DMA use)."""
    return bass.AP(tensor=ap.tensor, offset=ap.offset, ap=[[0, p], *ap.ap[1:]])


def _build_nc():
    nc = bass.Bass("TRN2", target_bir_lowering=False)
    nc._drain_scratch = nc.sbuf_tensor("drainscr", [1, 1], F32).__enter__()

    imt_d = nc.dram_tensor("imt", [KC, 128, SHARD], F8, kind="ExternalInput")
    st_d = nc.dram_tensor("st", [KC, 128, JL], F8, kind="ExternalInput")
    gmask_d = nc.dram_tensor("gmask", [PG, PG], BF16, kind="ExternalInput")
    onesb_d = nc.dram_tensor("onesb", [PG, NIMG_G], BF16, kind="ExternalInput")
    maskjl_d = nc.dram_tensor("maskjl", [1, JL], BF16, kind="ExternalInput")
    lse_d = nc.dram_tensor("lseout", [N, JCAP], F32, kind="ExternalOutput")

    with tile.TileContext(nc) as tc:
        with (
            tc.tile_pool(name="persist", bufs=1) as pp,
            tc.tile_pool(name="work", bufs=int(os.environ.get("K_WPB", "2"))) as wp,
            tc.tile_pool(name="fb", bufs=WIN + 1) as fbp,
            tc.tile_pool(name="scr1", bufs=1) as scrp,
            tc.tile_pool(name="post", bufs=1) as postp,
            tc.tile_pool(name="small", bufs=3) as sp,
            tc.tile_pool(name="bps", bufs=1, space="PSUM") as bpool,
            tc.tile_pool(name="hps", bufs=2, space="PSUM") as hpool,
            tc.tile_pool(name="pqps", bufs=2, space="PSUM") as pqpool,
        ):
            imt = pp.tile([128, KC, IRPAD], BF16)
            st = pp.tile([128, KC, JL], BF16)
            gmask = pp.tile([PG, PG], BF16)
            onesb = pp.tile([PG, NIMG_G], BF16)
            g_all = pp.tile([PG, NG, PG], BF16)
            pq_all = pp.tile([128, 2, 2, JL], F32)   # [row, itile, P/Q, jl]
            cn_b = pp.tile([128, JL], F32)
            mask_b = pp.tile([128, JL], BF16)

            # images arrive sharded 1/8 per core; AllGather over the chip
            # interconnect instead of shipping 8 replicas through the host
            # tunnel (that replication dominated dispatch wall time).
            with (
                tc.tile_pool(name="ccdr", bufs=1, space="DRAM") as ccp,
                tc.tile_pool(name="ldq", bufs=3) as ldp,
            ):
                inb = ccp.tile([KC, 128, SHARD], F8)
                outb = ccp.tile([NCORES, KC, 128, SHARD], F8,
                                addr_space="Shared")
                nc.gpsimd.dma_start(inb[:, :, :], imt_d[:, :, :])
                nc.gpsimd.collective_compute(
                    "AllGather", ALU.bypass,
                    replica_groups=[list(range(NCORES))],
                    ins=[inb.opt()], outs=[outb.opt()])
                for r in range(NCORES):
                    for kc in range(KC):
                        stg = ldp.tile([128, SHARD], F8, tag="stg")
                        nc.sync.dma_start(out=stg, in_=outb[r, kc])
                        nc.scalar.copy(
                            imt[:, kc, r * SHARD:(r + 1) * SHARD], stg)
                for kc in range(KC):
                    stg2 = ldp.tile([128, JL], F8, tag="stg2")
                    nc.sync.dma_start(out=stg2, in_=st_d[kc])
                    nc.scalar.copy(st[:, kc, :], stg2)
            nc.sync.dma_start(out=gmask, in_=gmask_d[:, :])
            nc.sync.dma_start(out=onesb, in_=onesb_d[:, :])
            nc.sync.dma_start(out=mask_b, in_=_bcast_part(maskjl_d[0:1, :], 128))

            # ---- caption word norms cn[jl] = ||s_word||  (from masked sT) ----
            cn_sb = pp.tile([1, JL], F32)
            sq0 = postp.tile([128, JL], F32, tag="pA")
            sq1 = postp.tile([128, JL], F32, tag="pB")
            nc.vector.tensor_mul(sq0, st[:, 0, :], st[:, 0, :])
            nc.vector.tensor_mul(sq1, st[:, 1, :], st[:, 1, :])
            ones128 = pp.tile([128, 1], F32)
            nc.vector.memset(ones128, 1.0)
            for c0, cw in CHUNKS:
                cnps = pqpool.tile([1, 512], F32, tag="pq")
                nc.tensor.matmul(cnps[:, :cw], ones128, sq0[:, c0:c0 + cw],
                                 start=True, stop=False)
                nc.tensor.matmul(cnps[:, :cw], ones128, sq1[:, c0:c0 + cw],
                                 start=False, stop=True)
                nc.scalar.sqrt(cn_sb[0:1, c0:c0 + cw], cnps[:, :cw])
            # keep masked columns finite: cn = max(cn, 1e-6)
            nc.vector.tensor_scalar_max(cn_sb, cn_sb, 1e-6)
            with tc.tile_pool(name="drbnc", bufs=1, space="DRAM") as drp:
                cn_dr = drp.tile([1, JL], F32)
                nc.sync.dma_start(out=cn_dr[:, :], in_=cn_sb[:, :])
                nc.sync.dma_start(out=cn_b, in_=_bcast_part(cn_dr[0:1, :], 128))

            # ---- per-group Gram matrices (block-diag masked) ----
            for g in range(NG):
                gsl = slice(g * PG, (g + 1) * PG)
                gps = pqpool.tile([PG, PG], F32, tag="pq")
                for kc in range(KC):
                    nc.tensor.matmul(gps, imt[:, kc, gsl], imt[:, kc, gsl],
                                     start=(kc == 0), stop=(kc == KC - 1))
                nc.vector.tensor_mul(g_all[:, g, :], gps, gmask)

            # ---- main pipeline: windows of 4 groups ----
            for w in range((NG + WIN - 1) // WIN):
                gset = [g for g in range(w * WIN, min((w + 1) * WIN, NG))]
                fts = {}
                for g in gset:
                    gsl = slice(g * PG, (g + 1) * PG)
                    bps = bpool.tile([PG, JL], F32, tag="B")
                    for c0, cw in CHUNKS:
                        for kc in range(KC):
                            nc.tensor.matmul(bps[:, c0:c0 + cw], imt[:, kc, gsl],
                                             st[:, kc, c0:c0 + cw],
                                             start=(kc == 0), stop=(kc == KC - 1))

                    Rt = wp.tile([PG, JL], BF16, tag="R")
                    Bc = wp.tile([PG, JL], BF16, tag="Bc")
                    nc.scalar.activation(Rt, bps, AF.Lrelu, alpha=0.1)   # ACT
                    _bceng = nc.scalar.copy if os.environ.get("K_BC", "v") == "s" else nc.vector.tensor_copy
                    _bceng(Bc, bps)

                    St = wp.tile([PG, JL], BF16, tag="S")
                    nc.scalar.square(St, Rt)                             # ACT
                    n2 = sp.tile([PG, JCAP], F32, tag="n2")
                    nc.vector.tensor_reduce(
                        n2, St.rearrange("p (j l) -> p j l", l=L),
                        axis=mybir.AxisListType.X, op=ALU.add)           # DVE
                    n1 = sp.tile([PG, JCAP], F32, tag="n1")
                    nc.scalar.sqrt(n1, n2)                               # ACT small
                    nc.vector.tensor_scalar_add(n1, n1, EPS)             # DVE small
                    inv = sp.tile([PG, JCAP], F32, tag="inv")
                    nc.vector.reciprocal(inv, n1)                        # DVE small

                    M1 = wp.tile([PG, JL], BF16, tag="M1")
                    _m1eng = nc.vector if os.environ.get("K_M1", "g") == "v" else nc.gpsimd
                    _m1eng.tensor_tensor(
                        M1.rearrange("p (j l) -> p j l", l=L),
                        Rt.rearrange("p (j l) -> p j l", l=L),
                        _bcast_inner(inv[:, :], L), op=ALU.mult)
                    Et = wp.tile([PG, JL], BF16, tag="E")
                    nc.scalar.activation(Et, M1, AF.Exp, scale=LSM)      # ACT

                    F1 = fbp.tile([PG, JL], BF16, tag="F1")
                    _f1eng = nc.vector if os.environ.get("K_F1", "g") == "v" else nc.gpsimd
                    _f1eng.tensor_mul(F1, Et, Bc)
                    F2 = fbp.tile([PG, JL], BF16, tag="F2")
                    for c0, cw in CHUNKS:
                        hps = hpool.tile([PG, 512], F32, tag="H")
                        nc.tensor.matmul(hps[:, :cw], g_all[:, g, :],
                                         Et[:, c0:c0 + cw], start=True, stop=True)
                        nc.vector.tensor_mul(F2[:, c0:c0 + cw],
                                             Et[:, c0:c0 + cw], hps[:, :cw])  # DVE
                    fts[g] = (F1, F2)

                # PQ reduce for the window: 32-aligned psum slots per group
                scr = scrp.tile([99, 2, JL], F32, tag="scr")
                for c0, cw in PQCH:
                    pqa = pqpool.tile([99, 2, 256], F32, tag="pq")
                    for qi, g in enumerate(gset):
                        for pqi in range(2):
                            nc.tensor.matmul(
                                pqa[32 * qi:32 * qi + NIMG_G, pqi, :cw],
                                onesb, fts[g][pqi][:, c0:c0 + cw],
                                start=True, stop=True,
                                tile_position=(0, 32 * qi))
                    nc.scalar.copy(scr[:, :, c0:c0 + cw], pqa[:, :, :cw])  # ACT
                # scatter rows: image 3g+b lives at scr[32*(g%WIN)+b]
                for qi, g in enumerate(gset):
                    nimg = NIMG_G if g < NG - 1 else N - NIMG_G * (NG - 1)
                    b = 0
                    while b < nimg:
                        row = g * NIMG_G + b
                        it, r0 = row // 128, row % 128
                        nrun = min(nimg - b, 128 - r0)
                        nc.sync.dma_start(
                            out=pq_all[r0:r0 + nrun, it, :, :],
                            in_=scr[32 * qi + b:32 * qi + b + nrun, :, :])
                        b += nrun

            # ---- post stage: sim -> exp -> masked LSE ----
            for it in range(2):
                qa = postp.tile([128, JL], F32, tag="pA")
                qb = postp.tile([128, JL], F32, tag="pB")
                nc.scalar.sqrt(qa, pq_all[:, it, 1, :])              # q = sqrt(Q^2)
                nc.vector.tensor_mul(qa, qa, cn_b)                   # q*cn in place
                nc.vector.reciprocal(qb, qa)                         # 1/(q*cn)
                nc.vector.tensor_mul(qb, pq_all[:, it, 0, :], qb)    # sim in place
                nc.scalar.activation(qa, qb, AF.Exp, scale=LLSE)
                nc.vector.tensor_mul(qa, qa, mask_b)                 # masked exp
                ssum = sp.tile([128, JCAP], F32, tag="ssum")
                nc.vector.tensor_reduce(
                    ssum, qa.rearrange("p (j l) -> p j l", l=L),
                    axis=mybir.AxisListType.X, op=ALU.add)
                lse = sp.tile([128, JCAP], F32, tag="lse")
                nc.scalar.activation(lse, ssum, AF.Ln)
                nc.sync.dma_start(out=lse_d[it * 128:(it + 1) * 128, :], in_=lse)

    return nc


def kernel(im, s, cap_lens):
    im = np.asarray(im, np.float32)
    s = np.asarray(s, np.float32)
    cap_lens = np.asarray(cap_lens, np.int32)

    # host prep: mask padded words, transpose to (d, rows), pad ir, cast bf16
    wmask = (np.arange(L)[None, :] < cap_lens[:, None])          # (N, L)
    s_m = s * wmask[:, :, None].astype(np.float32)
    imt_full = np.zeros((D, IRPAD), np.float32)
    imt_full[:, :N * R] = im.reshape(N * R, D).T
    imt_bf = imt_full.astype(ml_dtypes.float8_e4m3)

    gmask = np.kron(np.eye(NIMG_G, dtype=np.float32),
                    np.ones((R, R), np.float32)).astype(ml_dtypes.bfloat16)
    onesb = np.kron(np.eye(NIMG_G, dtype=np.float32),
                    np.ones((R, 1), np.float32)).astype(ml_dtypes.bfloat16)

    in_maps = []
    for c in range(NCORES):
        js = slice(c * JCAP, (c + 1) * JCAP)
        stc = s_m[js].reshape(JL, D).T                            # (256, 1600)
        stc = np.ascontiguousarray(
            stc.reshape(KC, 128, JL)).astype(ml_dtypes.float8_e4m3)
        mjl = wmask[js].reshape(1, JL).astype(ml_dtypes.bfloat16)
        imtc = np.ascontiguousarray(
            imt_bf[:, c * SHARD:(c + 1) * SHARD].reshape(KC, 128, SHARD))
        in_maps.append({"imt": imtc, "st": stc, "gmask": gmask,
                        "onesb": onesb, "maskjl": mjl})

    _NC_CACHE["in_maps"] = in_maps
    if "nc" not in _NC_CACHE:
        nc = _build_nc()
        patched = _split_multiwaits(nc)
        nc.to_json_bytes = lambda: patched
        _NC_CACHE["nc"] = nc
    res = run_bass_kernel_spmd(_NC_CACHE["nc"], in_maps,
                               core_ids=list(range(NCORES)))
    outs = res.results if hasattr(res, "results") else res

    scores = np.concatenate(
        [o["lseout"].astype(np.float64) / LLSE for o in outs], axis=1)  # (256,256)

    d = np.diag(scores)
    cs = np.maximum(MARGIN + scores - d[:, None], 0.0)
    ci = np.maximum(MARGIN + scores - d[None, :], 0.0)
    np.fill_diagonal(cs, 0.0)
    np.fill_diagonal(ci, 0.0)
    return np.float32(cs.sum() + ci.sum())

